# revision 2
# baseline (speedup 1.0000x reference)
"""2-layer GAT (GATConv + elu, masked output) on 8 Trainium2 NeuronCores.

v2 architecture (dst-partitioned, bf16, indirect gathers):
- Global node permutation: nodes sorted by in-degree, snake-dealt to 8
  cores; within a core nodes stay degree-sorted so 128-node dst blocks
  have near-uniform degree (tight slot padding).
- Layer tables in DRAM, bf16 rows: t1 row = [hx(64)|alpha_s|1.0] (66
  elems, 132B); t2 row = [hx2(32)|alpha_s2|1.0|alpha_d2] (36 elems).
- Edge slots are dst-partitioned: block b holds dst nodes b*128..+128 on
  partitions; cols_b = max in-degree in the block (shared across cores).
  Slot (p, j) gathers its src row via indirect_dma_start (one int32
  offset per partition per column; no 32k chunk limit, no one-hot S).
- ex = exp(leaky(alpha)) per slot; U|D accumulate via a column tree-add
  of row*ex (the 1.0 column yields the denominator). alpha_d comes from
  a resident per-block column (layer1, from the table build) or the
  self-loop slot pinned at column 0 (layer2).
- Layer 2 computes only masked nodes (h2[mask]), ~5.5x fewer edges.
- alpha_e terms are host-precomputed (edge_attr @ We.a_e fold).
"""

import sys
from dataclasses import dataclass

import numpy as np

sys.path.insert(0, "/opt/trn_rl_repo")

AE_PAD = -100.0  # alpha_e sentinel for pad slots -> ex == 0


@dataclass(frozen=True)
class Cfg:
    N: int = 100000
    E: int = 1600000
    F: int = 64
    C: int = 64
    O: int = 32
    ED: int = 16
    NCORES: int = 8
    EL1: int = 66    # t1 row elems: hx(64) | a_s | 1.0
    EL2: int = 36    # t2 row elems: hx2(32) | a_s | 1.0 | a_d

    @property
    def NPC(self):
        return self.N // self.NCORES

    @property
    def NB(self):
        return (self.NPC + 127) // 128

    @property
    def NPCP(self):
        return self.NB * 128

    @property
    def NTOT(self):
        return self.NCORES * self.NPCP


CFG_FULL = Cfg()


# ===================================================================== host
def prepare(cfg, edge_index, edge_attr, mask, We1, a_e1, We2, a_e2):
    """Permute nodes, build per-core dst-partitioned slot streams."""
    c = cfg
    src0 = edge_index[0].astype(np.int64)
    dst0 = edge_index[1].astype(np.int64)

    deg = np.bincount(dst0, minlength=c.N)  # in-degree, no self loop

    # ---- global permutation: sort by degree desc, snake deal to cores
    order = np.argsort(-deg, kind="stable")       # node ids, deg desc
    rank = np.empty(c.N, np.int64)
    rank[order] = np.arange(c.N)
    core_of_rank = np.arange(c.N) % c.NCORES
    snake = (np.arange(c.N) // c.NCORES) % 2 == 1
    core_of_rank[snake] = c.NCORES - 1 - core_of_rank[snake]
    pos_in_core = np.zeros(c.N, np.int64)
    for ci in range(c.NCORES):
        m = core_of_rank == ci
        pos_in_core[m] = np.arange(m.sum())
    # node id -> (core, pos); pos preserves degree order
    node_core = core_of_rank[rank]
    node_pos = pos_in_core[rank]
    gid = node_core * c.NPCP + node_pos            # padded global row id

    # ---- alpha_e host fold: ae = edge_attr @ (We a_e)
    v1 = (We1.astype(np.float64) @ a_e1.astype(np.float64).reshape(-1)
          ).astype(np.float32)
    v2 = (We2.astype(np.float64) @ a_e2.astype(np.float64).reshape(-1)
          ).astype(np.float32)
    ae1_e = edge_attr.astype(np.float32) @ v1      # [E]
    ae2_e = edge_attr.astype(np.float32) @ v2

    # edges into masked nodes (layer 2 works only on these)
    mask = np.asarray(mask).astype(np.int64)
    um = np.unique(mask)
    is_m = np.zeros(c.N, bool)
    is_m[um] = True
    e2sel = is_m[dst0]
    src2 = src0[e2sel]
    dst2 = dst0[e2sel]
    ae2_sel = ae2_e[e2sel]
    deg2 = np.bincount(dst2, minlength=c.N)

    # ================= layer 1 slots: all nodes, self loop at col 0
    # per (core, block): cols_b = 1 + max per-node in-deg in block
    deg_pos = np.zeros((c.NCORES, c.NPCP), np.int64)
    deg_pos[node_core, node_pos] = deg
    blk_deg = deg_pos.reshape(c.NCORES, c.NB, 128)
    cols1 = 1 + blk_deg.max(axis=(0, 2))           # [NB] shared, >=1
    col0_1 = np.zeros(c.NB, np.int64)
    col0_1[1:] = np.cumsum(cols1)[:-1]
    COLS1 = int(cols1.sum())

    # slot position for each edge: dst (core, pos) -> block, partition;
    # rank within its dst = running count (order within dst arbitrary)
    e_core = node_core[dst0]
    e_blk = node_pos[dst0] // 128
    e_par = node_pos[dst0] % 128
    # rank among edges sharing the same dst
    sort_d = np.argsort(dst0, kind="stable")
    cnt = np.bincount(dst0, minlength=c.N)
    starts = np.zeros(c.N, np.int64)
    starts[1:] = np.cumsum(cnt)[:-1]
    erank = np.empty(c.E, np.int64)
    erank[sort_d] = np.arange(c.E) - starts[dst0[sort_d]]
    e_col = col0_1[e_blk] + 1 + erank              # col 0 = self loop

    idx1 = np.zeros((c.NCORES, 128, COLS1), np.int32)
    ae1 = np.full((c.NCORES, 128, COLS1), AE_PAD, np.float32)
    idx1[e_core, e_par, e_col] = gid[src0].astype(np.int32)
    ae1[e_core, e_par, e_col] = ae1_e
    # self loops: block b partition p -> own row, ae = 0
    own = (np.arange(c.NCORES)[:, None] * c.NPCP
           + np.arange(c.NPCP)[None, :])           # [NCORES, NPCP]
    own_blk = own.reshape(c.NCORES, c.NB, 128)
    for b in range(c.NB):
        idx1[:, :, col0_1[b]] = own_blk[:, b, :]
        ae1[:, :, col0_1[b]] = 0.0

    # ================= layer 2 slots: masked nodes only, owner core
    m_core = node_core[um]
    # per-core masked node lists, degree-sorted
    NM = np.zeros(c.NCORES, np.int64)
    m_pos = np.empty(um.size, np.int64)            # slot pos within core
    for ci in range(c.NCORES):
        sel = np.where(m_core == ci)[0]
        o = sel[np.argsort(-deg2[um[sel]], kind="stable")]
        m_pos[o] = np.arange(o.size)
        NM[ci] = o.size
    NB2 = int((NM.max() + 127) // 128)
    NM2P = NB2 * 128
    # node -> (l2 core, l2 pos); -1 if not masked
    l2pos = np.full(c.N, -1, np.int64)
    l2pos[um] = m_pos
    l2core = np.full(c.N, -1, np.int64)
    l2core[um] = m_core

    dp2 = np.zeros((c.NCORES, NM2P), np.int64)
    valid = np.zeros((c.NCORES, NM2P), bool)
    dp2[l2core[um], m_pos] = deg2[um]
    valid[l2core[um], m_pos] = True
    cols2 = 1 + dp2.reshape(c.NCORES, NB2, 128).max(axis=(0, 2))
    col0_2 = np.zeros(NB2, np.int64)
    col0_2[1:] = np.cumsum(cols2)[:-1]
    COLS2 = int(cols2.sum())

    e2_core = l2core[dst2]
    e2_blk = l2pos[dst2] // 128
    e2_par = l2pos[dst2] % 128
    sort_d2 = np.argsort(dst2, kind="stable")
    cnt2 = np.bincount(dst2, minlength=c.N)
    st2 = np.zeros(c.N, np.int64)
    st2[1:] = np.cumsum(cnt2)[:-1]
    er2 = np.empty(dst2.size, np.int64)
    er2[sort_d2] = np.arange(dst2.size) - st2[dst2[sort_d2]]
    e2_col = col0_2[e2_blk] + 1 + er2

    idx2 = np.zeros((c.NCORES, 128, COLS2), np.int32)
    ae2 = np.full((c.NCORES, 128, COLS2), AE_PAD, np.float32)
    idx2[e2_core, e2_par, e2_col] = gid[src2].astype(np.int32)
    ae2[e2_core, e2_par, e2_col] = ae2_sel
    # self loops at col 0 of each block (valid nodes only; pads point
    # at row 0 with AE_PAD so ex=0 and D=0 -> discarded rows)
    own2 = np.zeros((c.NCORES, NM2P), np.int64)
    own2[l2core[um], m_pos] = gid[um]
    ae2_self = np.where(valid, 0.0, AE_PAD).astype(np.float32)
    ob2 = own2.reshape(c.NCORES, NB2, 128)
    sb2 = ae2_self.reshape(c.NCORES, NB2, 128)
    for b in range(NB2):
        idx2[:, :, col0_2[b]] = ob2[:, b, :]
        ae2[:, :, col0_2[b]] = sb2[:, b, :]

    # output mapping: mask entry i -> (core, pos) of node mask[i]
    out_core = l2core[mask]
    out_pos = l2pos[mask]

    meta = dict(cols1=cols1, COLS1=COLS1, cols2=cols2, COLS2=COLS2,
                NB2=NB2, gid=gid, out_core=out_core, out_pos=out_pos)
    return dict(idx1=idx1, ae1=ae1, idx2=idx2, ae2=ae2), meta


# ===================================================================== bass
def build_program(cfg, meta):
    import concourse.bass as bass
    import concourse.tile as tile
    import concourse.mybir as mybir
    from concourse import bacc
    from contextlib import ExitStack

    c = cfg
    dt = mybir.dt
    AF = mybir.ActivationFunctionType
    ALU = mybir.AluOpType
    f32, bf16, i32 = dt.float32, dt.bfloat16, dt.int32
    cols1, COLS1 = meta["cols1"], meta["COLS1"]
    cols2, COLS2 = meta["cols2"], meta["COLS2"]
    NB2 = meta["NB2"]
    NTILE = c.NTOT // 128

    nc = bacc.Bacc("TRN2", target_bir_lowering=False, debug=False,
                   num_devices=c.NCORES)

    xTown = nc.dram_tensor("xTown", [c.F, c.NPCP], bf16,
                           kind="ExternalInput").ap()
    idx1 = nc.dram_tensor("idx1", [128, COLS1], i32,
                          kind="ExternalInput").ap()
    ae1 = nc.dram_tensor("ae1", [128, COLS1], bf16,
                         kind="ExternalInput").ap()
    idx2 = nc.dram_tensor("idx2", [128, COLS2], i32,
                          kind="ExternalInput").ap()
    ae2 = nc.dram_tensor("ae2", [128, COLS2], bf16,
                         kind="ExternalInput").ap()
    # Wa1: [F, 66] = [W1(64) | ws1 | zero(->1.0)]
    Wa1 = nc.dram_tensor("Wa1", [c.F, 66], bf16, kind="ExternalInput").ap()
    # Wa2: [C, 36] = [W2(32) | ws2 | zero(->1.0) | wd2 | pad]
    Wa2 = nc.dram_tensor("Wa2", [c.C, 36], bf16, kind="ExternalInput").ap()
    # P4 row layout (single partition): [b1(64) | b2(64) | a_d1(64) | pad]
    P4 = nc.dram_tensor("P4", [1, 256], f32, kind="ExternalInput").ap()

    t1own = nc.dram_tensor("t1own", [c.NPCP, c.EL1], bf16).ap()
    table1 = nc.dram_tensor("table1", [c.NTOT, c.EL1], bf16).ap()
    t2own = nc.dram_tensor("t2own", [c.NPCP, c.EL2], bf16).ap()
    table2 = nc.dram_tensor("table2", [c.NTOT, c.EL2], bf16).ap()
    h2own = nc.dram_tensor("h2own", [NB2 * 128, c.O], f32,
                           kind="ExternalOutput").ap()

    with tile.TileContext(nc) as tc, ExitStack() as ctx:
        consts = ctx.enter_context(tc.tile_pool(name="consts", bufs=1))
        sb = ctx.enter_context(tc.tile_pool(name="sb", bufs=3))
        gp = ctx.enter_context(tc.tile_pool(name="gath", bufs=4))
        pp = ctx.enter_context(tc.tile_pool(name="ps", bufs=2, space="PSUM"))

        # ---------------- constants
        ident = consts.tile([128, 128], f32, tag="ident")
        ones_t = consts.tile([128, 128], f32, tag="ones")
        nc.vector.memset(ones_t[:], 1.0)
        nc.gpsimd.affine_select(ident[:], ones_t[:], pattern=[[-1, 128]],
                                base=0, channel_multiplier=1,
                                compare_op=ALU.is_equal, fill=0.0)

        Wa1_s = consts.tile([c.F, 66], bf16, tag="wa1")
        nc.sync.dma_start(Wa1_s[:], Wa1)
        Wa2_s = consts.tile([c.C, 36], bf16, tag="wa2")
        nc.sync.dma_start(Wa2_s[:], Wa2)
        P4_s = consts.tile([1, 256], f32, tag="p4s")
        nc.sync.dma_start(P4_s[:], P4)
        b1bc = consts.tile([128, c.C], f32, tag="b1bc")
        nc.gpsimd.partition_broadcast(b1bc[:], P4_s[0:1, 0:c.C])
        b2bc = consts.tile([128, c.O], f32, tag="b2bc")
        nc.gpsimd.partition_broadcast(b2bc[:], P4_s[0:1, 64:64 + c.O])
        ad1row_f = consts.tile([128, c.C], f32, tag="ad1rf")
        nc.gpsimd.partition_broadcast(ad1row_f[:],
                                      P4_s[0:1, 128:128 + c.C])
        ad1row = consts.tile([128, c.C], bf16, tag="ad1row")
        nc.vector.tensor_copy(ad1row[:], ad1row_f[:])

        idx1_s = consts.tile([128, COLS1], i32, tag="idx1")
        nc.sync.dma_start(idx1_s[:], idx1)
        ae1_s = consts.tile([128, COLS1], bf16, tag="ae1")
        nc.sync.dma_start(ae1_s[:], ae1)
        idx2_s = consts.tile([128, COLS2], i32, tag="idx2")
        nc.sync.dma_start(idx2_s[:], idx2)
        ae2_s = consts.tile([128, COLS2], bf16, tag="ae2")
        nc.sync.dma_start(ae2_s[:], ae2)

        # ------- table1 build: own shard only (8-tile groups) + AllGather
        GROUP = 8
        for t0g in range(0, c.NB, GROUP):
            g = min(GROUP, c.NB - t0g)
            lhsT = sb.tile([c.F, GROUP * 128], bf16, tag="b1L")
            nc.sync.dma_start(lhsT[:, 0:g * 128],
                              xTown[:, t0g * 128:(t0g + g) * 128])
            roww = sb.tile([128, GROUP * c.EL1], bf16, tag="b1R")
            for k in range(g):
                hp = pp.tile([128, 512], f32, tag="work")
                nc.tensor.matmul(hp[:, 0:66],
                                 lhsT[:, k * 128:(k + 1) * 128],
                                 Wa1_s[:, 0:66], start=True, stop=True)
                nc.vector.tensor_copy(
                    roww[:, k * c.EL1:k * c.EL1 + 65], hp[:, 0:65])
                nc.vector.memset(
                    roww[:, k * c.EL1 + 65:(k + 1) * c.EL1], 1.0)
            dview = t1own[t0g * 128:(t0g + g) * 128, :].rearrange(
                "(k p) e -> p k e", p=128)
            sview = roww[:, 0:g * c.EL1].rearrange(
                "p (k e) -> p k e", e=c.EL1)
            nc.sync.dma_start(dview, sview)
        if c.NCORES > 1:
            nc.gpsimd.collective_compute(
                "AllGather", mybir.AluOpType.bypass,
                replica_groups=[list(range(c.NCORES))],
                ins=[t1own[:, :].opt()], outs=[table1[:, :].opt()])
            t1ap = table1
        else:
            t1ap = t1own

        # ---------------- generic edge pass
        def edge_pass(layer, nb, colsv, tableap, el, hw, idx_s, ae_s,
                      adcol_elem, finalize):
            cbmax = int(max(colsv))
            col0 = 0
            for b in range(nb):
                cb = int(colsv[b])
                G = gp.tile([128, cbmax * el], bf16, tag=f"G{layer}")
                for j in range(cb):
                    nc.gpsimd.indirect_dma_start(
                        out=G[:, j * el:(j + 1) * el],
                        out_offset=None,
                        in_=tableap,
                        in_offset=bass.IndirectOffsetOnAxis(
                            ap=idx_s[:, col0 + j:col0 + j + 1], axis=0),
                    )
                G3 = G[:].rearrange("p (n e) -> p n e", e=el)
                # alpha: u = a_s(gathered) + ae + a_d(block)
                u = sb.tile([128, cbmax], f32, tag="u")
                nc.vector.tensor_tensor(
                    u[:, 0:cb], G3[:, 0:cb, hw],
                    ae_s[:, col0:col0 + cb], op=ALU.add)
                # alpha_d from the self-loop row (col 0):
                adc = sb.tile([128, 1], f32, tag="adc")
                if layer == 1:
                    # ad[p] = hx_self . a_d  (reduce of col0 hx * ad1row)
                    adt = sb.tile([128, c.C], f32, tag="adt")
                    nc.vector.tensor_tensor(
                        adt[:], G[:, 0:c.C], ad1row[:], op=ALU.mult)
                    nc.vector.tensor_reduce(
                        adc[:], adt[:], axis=mybir.AxisListType.X,
                        op=ALU.add)
                else:
                    nc.vector.tensor_copy(
                        adc[:], G3[:, 0:1, adcol_elem])
                nc.vector.tensor_tensor(
                    u[:, 0:cb], u[:, 0:cb],
                    adc[:].to_broadcast([128, cb]), op=ALU.add)
                # leaky relu + exp
                nc.vector.scalar_tensor_tensor(
                    u[:, 0:cb], u[:, 0:cb], 0.2, u[:, 0:cb],
                    op0=ALU.mult, op1=ALU.max)
                exb = sb.tile([128, cbmax], bf16, tag="exb")
                nc.scalar.activation(exb[:, 0:cb], u[:, 0:cb], AF.Exp)
                # vals = G * ex
                vals = sb.tile([128, cbmax * el], bf16, tag="vals")
                nc.vector.tensor_tensor(
                    vals[:].rearrange("p (n e) -> p n e", e=el)[:, 0:cb, :],
                    G3[:, 0:cb, :],
                    exb[:, 0:cb].to_broadcast([128, cb, el]), op=ALU.mult)
                # tree reduce over columns -> acc fp32
                w = cb
                while w > 2:
                    h = w // 2
                    nc.vector.tensor_tensor(
                        vals[:, 0:h * el], vals[:, 0:h * el],
                        vals[:, (w - h) * el:w * el], op=ALU.add)
                    w = w - h
                acc = sb.tile([128, el], f32, tag="acc")
                if w == 2:
                    nc.vector.tensor_tensor(
                        acc[:], vals[:, 0:el], vals[:, el:2 * el],
                        op=ALU.add)
                else:
                    nc.vector.tensor_copy(acc[:], vals[:, 0:el])
                finalize(b, acc)
                col0 += cb

        # ---------------- layer-1 finalize: h1 -> t2own row + DRAM
        def fin1(b, acc):
            dcl = sb.tile([128, 1], f32, tag="dcl")
            nc.vector.tensor_scalar(dcl[:], acc[:, 65:66], 1e-30, None,
                                    op0=ALU.max)
            rec = sb.tile([128, 1], f32, tag="rec")
            nc.vector.reciprocal(rec[:], dcl[:])
            h = sb.tile([128, c.C], f32, tag="hfin")
            nc.vector.tensor_tensor(h[:], acc[:, 0:c.C],
                                    rec[:].to_broadcast([128, c.C]),
                                    op=ALU.mult)
            nc.vector.tensor_tensor(h[:], h[:], b1bc[:], op=ALU.add)
            # elu
            m = sb.tile([128, c.C], f32, tag="melu")
            nc.vector.tensor_scalar(m[:], h[:], 0.0, None, op0=ALU.min)
            nc.scalar.activation(m[:], m[:], AF.Exp)
            r = sb.tile([128, c.C], f32, tag="relu")
            nc.vector.tensor_scalar(r[:], h[:], 0.0, None, op0=ALU.max)
            nc.vector.scalar_tensor_tensor(h[:], m[:], -1.0, r[:],
                                           op0=ALU.add, op1=ALU.add)
            # transpose h -> [C, 128] bf16, then build t2own rows
            hps = pp.tile([128, 512], f32, tag="work")
            nc.tensor.transpose(hps[0:c.C, 0:128], h[:], ident[:])
            h1T = sb.tile([c.C, 128], bf16, tag="h1T")
            nc.vector.tensor_copy(h1T[:], hps[0:c.C, 0:128])
            hp2 = pp.tile([128, 512], f32, tag="work")
            nc.tensor.matmul(hp2[:, 0:36], h1T[:], Wa2_s[:],
                             start=True, stop=True)
            row = sb.tile([128, c.EL2], bf16, tag="t2R")
            nc.vector.tensor_copy(row[:, 0:33], hp2[:, 0:33])
            nc.vector.memset(row[:, 33:34], 1.0)
            nc.vector.tensor_copy(row[:, 34:36], hp2[:, 34:36])
            nc.sync.dma_start(t2own[b * 128:(b + 1) * 128, :], row[:])

        edge_pass(1, c.NB, cols1, t1ap[:, :], c.EL1, 64, idx1_s, ae1_s,
                  None, fin1)

        # ---------------- allgather t2own -> table2
        if c.NCORES > 1:
            nc.gpsimd.collective_compute(
                "AllGather", mybir.AluOpType.bypass,
                replica_groups=[list(range(c.NCORES))],
                ins=[t2own[:, :].opt()], outs=[table2[:, :].opt()])
            t2 = table2
        else:
            t2 = t2own

        # ---------------- layer-2 finalize: h2 -> DRAM
        def fin2(b, acc):
            dcl = sb.tile([128, 1], f32, tag="dcl")
            nc.vector.tensor_scalar(dcl[:], acc[:, 33:34], 1e-30, None,
                                    op0=ALU.max)
            rec = sb.tile([128, 1], f32, tag="rec")
            nc.vector.reciprocal(rec[:], dcl[:])
            h = sb.tile([128, c.O], f32, tag="h2fin")
            nc.vector.tensor_tensor(h[:], acc[:, 0:c.O],
                                    rec[:].to_broadcast([128, c.O]),
                                    op=ALU.mult)
            nc.vector.tensor_tensor(h[:], h[:], b2bc[:], op=ALU.add)
            m = sb.tile([128, c.O], f32, tag="melu2")
            nc.vector.tensor_scalar(m[:], h[:], 0.0, None, op0=ALU.min)
            nc.scalar.activation(m[:], m[:], AF.Exp)
            r = sb.tile([128, c.O], f32, tag="relu2")
            nc.vector.tensor_scalar(r[:], h[:], 0.0, None, op0=ALU.max)
            nc.vector.scalar_tensor_tensor(h[:], m[:], -1.0, r[:],
                                           op0=ALU.add, op1=ALU.add)
            nc.sync.dma_start(h2own[b * 128:(b + 1) * 128, :], h[:])

        edge_pass(2, NB2, cols2, t2[:, :], c.EL2, 32, idx2_s, ae2_s,
                  34, fin2)

    nc.compile()
    return nc


# ===================================================================== glue
def make_in_maps(cfg, inputs, streams, meta):
    import ml_dtypes
    c = cfg
    gid = meta["gid"]
    x = np.asarray(inputs["x"], np.float32)
    xp = np.zeros((c.NTOT, c.F), np.float32)
    xp[gid] = x
    xT = np.ascontiguousarray(xp.T).astype(ml_dtypes.bfloat16)
    xTowns = [np.ascontiguousarray(xT[:, ci * c.NPCP:(ci + 1) * c.NPCP])
              for ci in range(c.NCORES)]

    W1 = np.asarray(inputs["W1"], np.float32)
    a_s1 = np.asarray(inputs["a_s1"], np.float32).reshape(-1)
    a_d1 = np.asarray(inputs["a_d1"], np.float32).reshape(-1)
    W2 = np.asarray(inputs["W2"], np.float32)
    a_s2 = np.asarray(inputs["a_s2"], np.float32).reshape(-1)
    a_d2 = np.asarray(inputs["a_d2"], np.float32).reshape(-1)

    Wa1 = np.zeros((c.F, 66), np.float32)
    Wa1[:, 0:64] = W1
    Wa1[:, 64] = W1 @ a_s1
    Wa2 = np.zeros((c.C, 36), np.float32)
    Wa2[:, 0:32] = W2
    Wa2[:, 32] = W2 @ a_s2
    Wa2[:, 34] = W2 @ a_d2      # alpha_d2 = hx2 . a_d2 = h1 . (W2 a_d2)
    P4 = np.zeros((1, 256), np.float32)
    P4[0, 0:64] = np.asarray(inputs["b1"], np.float32).reshape(-1)
    P4[0, 64:96] = np.asarray(inputs["b2"], np.float32).reshape(-1)
    P4[0, 128:192] = a_d1       # hx-space a_d1 (C == F == 64)

    base = {
        "Wa1": Wa1.astype(ml_dtypes.bfloat16),
        "Wa2": Wa2.astype(ml_dtypes.bfloat16),
        "P4": P4,
    }
    in_maps = []
    for ci in range(c.NCORES):
        m = dict(base)
        m["xTown"] = xTowns[ci]
        m["idx1"] = streams["idx1"][ci]
        m["ae1"] = streams["ae1"][ci].astype(ml_dtypes.bfloat16)
        m["idx2"] = streams["idx2"][ci]
        m["ae2"] = streams["ae2"][ci].astype(ml_dtypes.bfloat16)
        in_maps.append(m)
    return in_maps


def assemble_output(cfg, results, meta):
    out_core = meta["out_core"]
    out_pos = meta["out_pos"]
    h2 = np.stack([r["h2own"] for r in results])   # [NCORES, NB2*128, O]
    return np.ascontiguousarray(h2[out_core, out_pos])


_CACHE = {}


def run_sharded(cfg, inputs):
    from concourse import bass_utils
    streams, meta = prepare(
        cfg, np.asarray(inputs["edge_index"]),
        np.asarray(inputs["edge_attr"], np.float32),
        np.asarray(inputs["mask"]),
        np.asarray(inputs["We1"], np.float32),
        np.asarray(inputs["a_e1"], np.float32),
        np.asarray(inputs["We2"], np.float32),
        np.asarray(inputs["a_e2"], np.float32))
    key = (cfg, meta["COLS1"], meta["COLS2"], meta["NB2"])
    if key not in _CACHE:
        _CACHE[key] = build_program(cfg, meta)
    nc = _CACHE[key]
    in_maps = make_in_maps(cfg, inputs, streams, meta)
    res = bass_utils.run_bass_kernel_spmd(
        nc, in_maps, core_ids=list(range(cfg.NCORES)))
    return assemble_output(cfg, res.results, meta), res, meta, in_maps


def kernel(**inputs) -> np.ndarray:
    out, _, _, _ = run_sharded(CFG_FULL, inputs)
    return out
